# revision 1
# baseline (speedup 1.0000x reference)
# Trainium2 Bass kernel for nn_FLAT_21242908246425 (2-layer relative-position
# transformer, B=16 S=512 D=256 H=8 DK=32 DFF=1024, V=8000 BV=50000 NL=25).
#
# Sharding: data-parallel over batch — 8 cores x 2 sequences each. No
# collectives. Each core runs the full network on its 2 sequences.
#
# Device layout: activations feature-major [D, N] in float32r (full-rate PE).
# Attention computes S^T = K Q^T per (seq, head) so the PV matmul needs no
# transpose of the softmax output. The relative-position term
# qrel[i, c(j-i)] is decomposed (after subtracting the softmax-invariant
# qrel[i,0]) into a 31-diagonal band + right-saturation, realized by a
# skewed DMA read from a pre-zeroed DRAM scratch plus K=1 column-restricted
# correction matmuls. Softmax Z comes free from a ones-column appended to V.
import os
import sys

import numpy as np

if "/opt/trn_rl_repo" not in sys.path:
    sys.path.insert(0, "/opt/trn_rl_repo")

B, S, D, H, DK, L, DFF, MAXREL, NL = 16, 512, 256, 8, 32, 2, 1024, 16, 25
V, BV = 8000, 50000
NCORES = 8
BL = B // NCORES          # sequences per core
N = BL * S                # local tokens (1024)
NCH = N // 128            # token chunks (8)
R = 2 * MAXREL + 1        # 33
SCALE = 1.0 / float(np.sqrt(DK))
NSCR = 4                  # rotating rel-scratch buffers

_cache = {}


def _build():
    import concourse.bass as bass
    import concourse.mybir as mybir
    import concourse.tile as tile
    from concourse import bacc
    from concourse.masks import make_identity
    from concourse.tile_rust import add_dep_helper

    dt = mybir.dt
    ALU = mybir.AluOpType
    AF = mybir.ActivationFunctionType
    f32, f32r = dt.float32, dt.float32r

    nc = bacc.Bacc("TRN2", target_bir_lowering=False, debug=False,
                   num_devices=NCORES)

    def inp(name, shape, dtype=dt.float32):
        return nc.declare_dram_parameter(name, list(shape), dtype,
                                         isOutput=False)

    cids_d = inp("cids", [128, NCH], dt.int32)
    bids_d = inp("bids", [128, NCH], dt.int32)
    cemb_d = inp("char_emb", [V, D])
    bemb_d = inp("bigram_emb", [BV, D])
    combW_d = inp("comb_W", [2 * D, D])
    combb_d = inp("comb_b", [D])
    peT_d = inp("peT", [D, S])
    Wq_d = inp("Wq", [L, D, D]); bq_d = inp("bq", [L, D])
    Wk_d = inp("Wk", [L, D, D]); bk_d = inp("bk", [L, D])
    Wv_d = inp("Wv", [L, D, D]); bv_d = inp("bv", [L, D])
    Wo_d = inp("Wo", [L, D, D]); bo_d = inp("bo", [L, D])
    rel_d = inp("rel_emb", [L, R, DK])
    ln1g_d = inp("ln1_g", [L, D]); ln1b_d = inp("ln1_b", [L, D])
    ln2g_d = inp("ln2_g", [L, D]); ln2b_d = inp("ln2_b", [L, D])
    W1_d = inp("W1", [L, D, DFF]); b1_d = inp("b1", [L, DFF])
    W2_d = inp("W2", [L, DFF, D]); b2_d = inp("b2", [L, D])
    clsW_d = inp("cls_W", [D, NL]); clsb_d = inp("cls_b", [NL])

    em_d = nc.declare_dram_parameter("emT", [NL, N], dt.float32,
                                     isOutput=True)
    scr = [nc.dram_tensor(f"scr{i}", [287, S], dt.float32)
           for i in range(NSCR)]

    with tile.TileContext(nc) as tc:
        import contextlib
        ctx = contextlib.ExitStack()
        with ctx:
            consts = ctx.enter_context(tc.tile_pool(name="consts", bufs=1))
            wpool = ctx.enter_context(tc.tile_pool(name="weights", bufs=1))
            act = ctx.enter_context(tc.tile_pool(name="acts", bufs=1))
            work = ctx.enter_context(tc.tile_pool(name="work", bufs=2))
            ffp = ctx.enter_context(tc.tile_pool(name="ffp", bufs=8))
            ptp = ctx.enter_context(tc.tile_pool(name="ptp", bufs=3))
            wtp = ctx.enter_context(tc.tile_pool(name="wtp", bufs=2))
            wtp2 = ctx.enter_context(tc.tile_pool(name="wtp2", bufs=3))
            rowp = ctx.enter_context(tc.tile_pool(name="rowp", bufs=6))
            rowp2 = ctx.enter_context(tc.tile_pool(name="rowp2", bufs=2))
            # PSUM: 8 banks total: mm(2) + stps(2) + cps(2) + lnps(1) + ff2(1)
            pp = ctx.enter_context(
                tc.tile_pool(name="pp", bufs=2, space="PSUM"))
            cpp = ctx.enter_context(
                tc.tile_pool(name="cpp", bufs=2, space="PSUM"))
            spp = ctx.enter_context(
                tc.tile_pool(name="spp", bufs=1, space="PSUM"))

            # ---------- constants ----------
            ident = consts.tile([128, 128], f32, tag="ident", name="ident")
            make_identity(nc, ident[:])

            ones_row_f = consts.tile([1, 128], f32, tag="ones_row_f", name="ones_row_f")
            nc.gpsimd.memset(ones_row_f[:], 1.0)
            ones_row = consts.tile([1, 128], f32r, tag="ones_row", name="ones_row")
            nc.vector.tensor_copy(ones_row[:], ones_row_f[:])
            ones_col_f = consts.tile([128, 1], f32, tag="ones_col_f", name="ones_col_f")
            nc.gpsimd.memset(ones_col_f[:], 1.0)
            ones_col = consts.tile([128, 1], f32r, tag="ones_col", name="ones_col")
            nc.vector.tensor_copy(ones_col[:], ones_col_f[:])

            onz_f = consts.tile([128, 2], f32, tag="onz_f", name="onz_f")
            nc.gpsimd.memset(onz_f[:, 0:1], 1.0)
            nc.gpsimd.memset(onz_f[:, 1:2], 0.0)
            onz_row = consts.tile([128, 2], f32r, tag="onz", name="onz")
            nc.vector.tensor_copy(onz_row[:], onz_f[:])

            zo_f = consts.tile([2, 128], f32, tag="zo_f", name="zo_f")
            nc.gpsimd.memset(zo_f[:], 0.0)
            nc.gpsimd.affine_select(
                out=zo_f[:], in_=zo_f[:], compare_op=ALU.not_equal,
                fill=1.0, base=0, pattern=[[0, 128]], channel_multiplier=1)
            zo_row = consts.tile([2, 128], f32r, tag="zo", name="zo")
            nc.vector.tensor_copy(zo_row[:], zo_f[:])

            eps_t = consts.tile([1, 1], f32, tag="eps", name="eps")
            nc.gpsimd.memset(eps_t[:], 1e-5)

            # M^T [33, 32]: row k, col r': 1 if k==r'+1, -1 if k==0
            mt_f = consts.tile([R, DK], f32, tag="mt_f", name="mt_f")
            nc.gpsimd.memset(mt_f[:], 0.0)
            nc.gpsimd.affine_select(
                out=mt_f[:], in_=mt_f[:], compare_op=ALU.not_equal,
                fill=1.0, base=-1, pattern=[[-1, DK]], channel_multiplier=1)
            nc.gpsimd.affine_select(
                out=mt_f[:], in_=mt_f[:], compare_op=ALU.not_equal,
                fill=-1.0, base=0, pattern=[[0, DK]], channel_multiplier=1)
            mt = consts.tile([R, DK], f32r, tag="mt", name="mt")
            nc.vector.tensor_copy(mt[:], mt_f[:])

            zt = consts.tile([128, S], f32, tag="zt", name="zt")
            nc.gpsimd.memset(zt[:], 0.0)
            for i in range(NSCR):
                nc.sync.dma_start(scr[i][0:128, :], zt[:])

            peT = []
            for m in range(2):
                t = consts.tile([128, S], f32, tag=f"peT{m}", name=f"peT{m}")
                nc.sync.dma_start(t[:], peT_d[m * 128:(m + 1) * 128, :])
                peT.append(t)

            def load_round(dram_ap, shape, tag, scale=1.0):
                raw = work.tile(list(shape), f32, tag="ldraw")
                nc.sync.dma_start(raw[:], dram_ap)
                t = wpool.tile(list(shape), f32r, tag=tag)
                if scale == 1.0:
                    nc.vector.tensor_copy(t[:], raw[:])
                else:
                    nc.scalar.mul(t[:], raw[:], scale)
                return t

            def load_col(dram_1d, n, tag, scale=1.0):
                cols = (n + 127) // 128
                p = min(n, 128)
                t = consts.tile([p, cols], f32, tag=tag)
                if n >= 128:
                    nc.sync.dma_start(
                        t[:], dram_1d.rearrange("(c p) -> p c", p=128))
                else:
                    nc.sync.dma_start(t[:], dram_1d.rearrange("(p o) -> p o", o=1))
                if scale != 1.0:
                    nc.scalar.mul(t[:], t[:], scale)
                return t

            def load_row_r(dram_1d, n, tag):
                raw = work.tile([1, n], f32, tag="ldrow", name="ldrow")
                nc.sync.dma_start(raw[:], dram_1d.rearrange("(o n) -> o n", o=1))
                t = consts.tile([1, n], f32r, tag=tag)
                nc.vector.tensor_copy(t[:], raw[:])
                return t

            combW = [load_round(combW_d[k * 128:(k + 1) * 128, :],
                                [128, D], f"combW{k}") for k in range(4)]
            combb = load_col(combb_d[:], D, "combb")
            clsW = []
            for k in range(2):
                craw = work.tile([128, 26], f32, tag="ldraw",
                                 name=f"clsraw{k}")
                nc.gpsimd.memset(craw[:], 0.0)
                nc.sync.dma_start(craw[:, 0:NL],
                                  clsW_d[k * 128:(k + 1) * 128, :])
                t = wpool.tile([128, 26], f32r, tag=f"clsW{k}",
                               name=f"clsW{k}")
                nc.vector.tensor_copy(t[:], craw[:])
                clsW.append(t)
            clsb = load_col(clsb_d[:], NL, "clsb")

            # ---------- embeddings -> xcatT [4][128, N] f32r ----------
            _xct = ["q0", "q1", "k0", "k1"]
            xc = [act.tile([128, N], f32r, tag=_xct[k], name=f"xc{k}")
                  for k in range(4)]
            cids = consts.tile([128, NCH], dt.int32, tag="cids", name="cids")
            bids = consts.tile([128, NCH], dt.int32, tag="bids", name="bids")
            nc.sync.dma_start(cids[:], cids_d[:])
            nc.sync.dma_start(bids[:], bids_d[:])
            for c in range(NCH):
                for tbl, (ids, emb) in enumerate(
                        [(cids, cemb_d), (bids, bemb_d)]):
                    g = work.tile([128, D], f32, tag="gath", name="gath")
                    nc.gpsimd.indirect_dma_start(
                        out=g[:], out_offset=None, in_=emb[:],
                        in_offset=bass.IndirectOffsetOnAxis(
                            ap=ids[:, c:c + 1], axis=0))
                    for dc in range(2):
                        ptt = pp.tile([128, 128], f32, tag="mm", name="mm")
                        nc.tensor.transpose(
                            ptt[:], g[:, dc * 128:(dc + 1) * 128], ident[:])
                        nc.scalar.copy(
                            xc[tbl * 2 + dc][:, c * 128:(c + 1) * 128],
                            ptt[:])

            # ---------- comb + pe -> xT [2][128, N] f32r ----------
            xT = [act.tile([128, N], f32r, tag=f"x0_{m}", name=f"x0_{m}") for m in range(2)]
            for m in range(2):
                for s in range(BL):
                    ps = pp.tile([128, S], f32, tag="mm", name="mm")
                    for k in range(4):
                        nc.tensor.matmul(
                            ps[:], combW[k][:, m * 128:(m + 1) * 128],
                            xc[k][:, s * S:(s + 1) * S],
                            start=(k == 0), stop=(k == 3))
                    nc.vector.scalar_tensor_tensor(
                        out=xT[m][:, s * S:(s + 1) * S], in0=ps[:],
                        scalar=combb[:, m:m + 1], in1=peT[m][:],
                        op0=ALU.add, op1=ALU.add)

            # ---------- layers ----------
            stage = int(os.environ.get("KERNEL_STAGE", "9"))
            skip_win = bool(os.environ.get("KERNEL_SKIP_WIN"))
            skip_scr = bool(os.environ.get("KERNEL_SKIP_SCR"))
            skip_d32 = bool(os.environ.get("KERNEL_SKIP_D32"))
            skip_pv = bool(os.environ.get("KERNEL_SKIP_PV"))
            skip_x = bool(os.environ.get("KERNEL_SKIP_X"))
            bh_idx = 0
            for l in range(L if stage >= 4 else min(L, 1)):
                if stage <= 1:
                    break
                Wq = [load_round(Wq_d[l, k * 128:(k + 1) * 128, :],
                                 [128, D], "Wq%d" % k, scale=SCALE)
                      for k in range(2)]
                Wk_ = [load_round(Wk_d[l, k * 128:(k + 1) * 128, :],
                                  [128, D], "Wkk%d" % k) for k in range(2)]
                Wv = [load_round(Wv_d[l, k * 128:(k + 1) * 128, :],
                                 [128, D], "Wv%d" % k) for k in range(2)]
                Wo = [load_round(Wo_d[l, k * 128:(k + 1) * 128, :],
                                 [128, D], "Wo%d" % k) for k in range(2)]
                W1 = [load_round(W1_d[l, k * 128:(k + 1) * 128, :],
                                 [128, DFF], "W1_%d" % k) for k in range(2)]
                W2 = [load_round(W2_d[l, k * 128:(k + 1) * 128, :],
                                 [128, D], "W2_%d" % k) for k in range(8)]
                bq = load_col(bq_d[l, :], D, "bqc", scale=SCALE)
                bk = load_col(bk_d[l, :], D, "bkc")
                bvc = load_round(bv_d[l, :].rearrange("(c p) -> p c", p=128),
                                 [128, 2], "bvc")
                boc = load_col(bo_d[l, :], D, "boc")
                b1c = load_col(b1_d[l, :], DFF, "b1c")
                b2c = load_col(b2_d[l, :], D, "b2c")
                ln1g = load_row_r(ln1g_d[l, :], D, "ln1g")
                ln1b = load_col(ln1b_d[l, :], D, "ln1b")
                ln2g = load_row_r(ln2g_d[l, :], D, "ln2g")
                ln2b = load_col(ln2b_d[l, :], D, "ln2b")

                # relM = rel_emb_eff^T @ M^T  [DK, 32]; r' col = delta'_{r'+1}
                rel_sb = load_round(rel_d[l, :, :], [R, DK], "rel",
                                    scale=float(np.sqrt(DK)))
                relM_ps = spp.tile([DK, DK], f32, tag="lnps", name="lnps")
                nc.tensor.matmul(relM_ps[:], rel_sb[:], mt[:],
                                 start=True, stop=True)
                relM = wpool.tile([DK, DK], f32r, tag="relM", name="relM")
                nc.scalar.copy(relM[:], relM_ps[:])
                bd_f = wpool.tile([128, 128], f32, tag="bdf", name="bdf")
                nc.gpsimd.memset(bd_f[:], 0.0)
                bd = wpool.tile([128, 128], f32r, tag="bd", name="bd")
                nc.vector.tensor_copy(bd[:], bd_f[:])
                for hq in range(4):
                    nc.vector.tensor_copy(
                        bd[hq * 32:(hq + 1) * 32,
                           hq * 32:(hq + 1) * 32], relM[:])

                # bo_eff = bo + Wo^T bv
                bv2 = []
                for k in range(2):
                    t = wpool.tile([128, 2], f32r, tag=f"bv2_{k}",
                                   name=f"bv2_{k}")
                    nc.vector.tensor_copy(t[:, 0:1], bvc[:, k:k + 1])
                    nc.vector.tensor_copy(t[:, 1:2], onz_row[:, 1:2])
                    bv2.append(t)
                boe = []
                for m in range(2):
                    pb = spp.tile([128, 2], f32, tag="lnps", name="lnps")
                    for k in range(2):
                        nc.tensor.matmul(
                            pb[:], Wo[k][:, m * 128:(m + 1) * 128],
                            bv2[k][:], start=(k == 0), stop=(k == 1))
                    t = wpool.tile([128, 1], f32, tag="boe%d" % m, name="boe%d" % m)
                    nc.vector.tensor_add(t[:], pb[:, 0:1], boc[:, m:m + 1])
                    boe.append(t)

                # QKV projections
                qt = [act.tile([128, N], f32r, tag=f"q{m}", name=f"q{m}")
                      for m in range(2)]
                kt = [act.tile([128, N], f32r, tag=f"k{m}", name=f"k{m}")
                      for m in range(2)]
                for m in range(2):
                    for s in range(BL):
                        psq = pp.tile([128, S], f32, tag="mm", name="mm")
                        for k in range(2):
                            nc.tensor.matmul(
                                psq[:], Wq[k][:, m * 128:(m + 1) * 128],
                                xT[k][:, s * S:(s + 1) * S],
                                start=(k == 0), stop=(k == 1))
                        nc.scalar.activation(
                            qt[m][:, s * S:(s + 1) * S], psq[:],
                            AF.Identity, bias=bq[:, m:m + 1])
                        psk = pp.tile([128, S], f32, tag="mm", name="mm")
                        for k in range(2):
                            nc.tensor.matmul(
                                psk[:], Wk_[k][:, m * 128:(m + 1) * 128],
                                xT[k][:, s * S:(s + 1) * S],
                                start=(k == 0), stop=(k == 1))
                        nc.scalar.activation(
                            kt[m][:, s * S:(s + 1) * S], psk[:],
                            AF.Identity, bias=bk[:, m:m + 1])

                # V token-major with ones column: vaug[c] [128, 8*33]
                vaug = []
                for c in range(NCH):
                    pv = pp.tile([128, D], f32, tag="mm", name="mm")
                    for k in range(2):
                        nc.tensor.matmul(
                            pv[:],
                            xT[k][:, c * 128:(c + 1) * 128], Wv[k][:],
                            start=(k == 0), stop=(k == 1))
                    va = act.tile([128, 272], f32r, tag=f"va{c}", name=f"va{c}")
                    nc.vector.tensor_copy(
                        va[:].rearrange("p (h c) -> p h c", c=34)[:, :, 0:32],
                        pv[:].rearrange("p (h c) -> p h c", c=32))
                    tail_view = va[:].rearrange(
                        "p (h c) -> p h c", c=34)[:, :, 32:34]
                    nc.vector.tensor_copy(
                        tail_view,
                        onz_row[:].unsqueeze(1).to_broadcast([128, 8, 2]))
                    vaug.append(va)

                # delta' for all heads: dp[s][hc] [128, S] f32r
                dp = [[None, None] for _ in range(BL)]
                for s in range(BL):
                    for hc in range(2):
                        pd = pp.tile([128, S], f32, tag="mm", name="mm")
                        nc.tensor.matmul(
                            pd[:], bd[:], qt[hc][:, s * S:(s + 1) * S],
                            start=True, stop=True)
                        t = act.tile([128, S], f32r, tag=f"dp{s}{hc}", name=f"dp{s}{hc}")
                        nc.scalar.copy(t[:], pd[:])
                        dp[s][hc] = t

                if stage <= 2:
                    break
                # attention
                ctxT = [[act.tile([128, S], f32r, tag=f"ctx{s}{m}", name=f"ctx{s}{m}")
                         for m in range(2)] for s in range(BL)]
                for s in range(BL):
                    for h in range(H):
                        hc, hl = h // 4, h % 4
                        sc = scr[bh_idx % NSCR]
                        bh_idx += 1
                        d32 = rowp2.tile([2, S], f32r, tag="d32", name="d32")
                        d32b = wtp.tile([128, S], f32r, tag="d32b", name="d32b")
                        if not skip_d32:
                            nc.sync.dma_start(
                                d32[0:1, :],
                                dp[s][hc][hl * 32 + 31:hl * 32 + 32, :])
                            nc.sync.dma_start(
                                d32[1:2, :].bitcast(f32), zt[0:1, :])
                            nc.gpsimd.partition_broadcast(
                                d32b[:].bitcast(f32),
                                d32[0:1, :].bitcast(f32),
                                channels=128)
                        if not skip_scr:
                            nc.sync.dma_start(
                                sc[128:159, :],
                                dp[s][hc][hl * 32:hl * 32 + 31, :]
                                .bitcast(f32))
                            nc.sync.dma_start(
                                sc[159:287, :], d32b[:].bitcast(f32))
                        sc_flat = sc[:].rearrange("a b -> (a b)")

                        cps = cpp.tile([34, S], f32, tag="cps", name="cps")
                        for u in range(4):
                            st = pp.tile([128, S], f32, tag="stps", name="stps")
                            w = 128 * u - 16
                            nc.tensor.matmul(
                                st[:],
                                kt[hc][hl * 32:(hl + 1) * 32,
                                       s * S + u * 128:
                                       s * S + u * 128 + 128],
                                qt[hc][hl * 32:(hl + 1) * 32,
                                       s * S:(s + 1) * S],
                                start=True, stop=True,
                                tile_position=(hl * 32, 0))
                            mmx = None
                            if u > 0 and not skip_d32 and not skip_x:
                                mmx = nc.vector.tensor_add(
                                    st[:, 0:w], st[:, 0:w],
                                    d32b[:, 0:w].bitcast(f32))
                            if not skip_win:
                                q0 = 16 if u == 0 else 0
                                q1 = 144 if u == 3 else 160
                                wn = q1 - q0
                                base = (159 * S + 128 * u - 16
                                        + q0 * (1 - S))
                                src = bass.AP(sc_flat.tensor, base,
                                              [[S, 128], [1 - S, wn]])
                                wt = wtp2.tile([128, 160], f32, tag="wt", name="wt")
                                nc.sync.dma_start(wt[:, 0:wn], src)
                                c0 = 128 * u - 16 + q0
                                addi = nc.vector.tensor_add(
                                    st[:, c0:c0 + wn], st[:, c0:c0 + wn],
                                    wt[:, 0:wn])
                                if mmx is not None:
                                    add_dep_helper(
                                        addi.ins, mmx.ins, sync=True,
                                        reason="band add after X accum")
                            pt = ptp.tile([128, S], f32r, tag="pt", name="pt")
                            nc.scalar.activation(pt[:], st[:], AF.Exp)
                            if not skip_pv:
                                nc.tensor.matmul(
                                    cps[:],
                                    vaug[s * 4 + u][:, h * 34:(h + 1) * 34],
                                    pt[:], start=(u == 0), stop=(u == 3),
                                    tile_position=(0, 0))
                        if skip_pv:
                            nc.vector.tensor_copy(
                                ctxT[s][hc][hl * 32:(hl + 1) * 32, :],
                                pt[0:32, :])
                        else:
                            rz = rowp2.tile([1, S], f32, tag="rz", name="rz")
                            nc.vector.reciprocal(rz[:], cps[32:33, :])
                            rzb = wtp.tile([32, S], f32, tag="rzb", name="rzb")
                            nc.gpsimd.partition_broadcast(
                                rzb[:], rz[:], channels=32)
                            nc.vector.tensor_tensor(
                                out=ctxT[s][hc][hl * 32:(hl + 1) * 32, :],
                                in0=cps[0:32, :], in1=rzb[:], op=ALU.mult)

                def layer_norm(u_tiles, g_row, b_col, out_aps):
                    psu = spp.tile([1, S], f32, tag="lnps", name="lnps")
                    for m in range(2):
                        nc.tensor.matmul(psu[:], ones_col[:],
                                         u_tiles[m][:],
                                         start=(m == 0), stop=(m == 1),
                                         tile_position=(0, 0))
                    mm = rowp.tile([1, S], f32, tag="lnrow", name="mmu")
                    nc.vector.tensor_scalar(
                        out=mm[:], in0=psu[:], scalar1=1.0 / D,
                        scalar2=None, op0=ALU.mult)
                    usq = [work.tile([128, S], f32r, tag="scr2", name="usq")
                           for _ in range(2)]
                    for m in range(2):
                        nc.scalar.square(usq[m][:], u_tiles[m][:])
                    psq2 = spp.tile([1, S], f32, tag="lnps", name="lnps")
                    for m in range(2):
                        nc.tensor.matmul(psq2[:], ones_col[:], usq[m][:],
                                         start=(m == 0), stop=(m == 1),
                                         tile_position=(0, 0))
                    mm2 = rowp.tile([1, S], f32, tag="lnrow", name="mm2")
                    nc.vector.tensor_tensor(out=mm2[:], in0=mm[:],
                                            in1=mm[:], op=ALU.mult)
                    var = rowp.tile([1, S], f32, tag="lnrow", name="var")
                    nc.vector.scalar_tensor_tensor(
                        out=var[:], in0=psq2[:], scalar=1.0 / D,
                        in1=mm2[:], op0=ALU.mult, op1=ALU.subtract)
                    sd = rowp.tile([1, S], f32, tag="lnrow", name="sd")
                    nc.scalar.activation(sd[:], var[:], AF.Sqrt,
                                         bias=eps_t[:])
                    invs = rowp.tile([1, S], f32, tag="lnrow", name="invs")
                    nc.vector.reciprocal(invs[:], sd[:])
                    invsr = rowp.tile([1, S], f32r, tag="lnrow", name="invsr")
                    nc.vector.tensor_copy(invsr[:], invs[:])
                    msn = rowp.tile([1, S], f32r, tag="lnrow", name="msn")
                    nc.vector.scalar_tensor_tensor(
                        out=msn[:], in0=mm[:], scalar=-1.0, in1=invs[:],
                        op0=ALU.mult, op1=ALU.mult)
                    for m in range(2):
                        sg = spp.tile([128, S], f32, tag="lnps", name="lnps")
                        nc.tensor.matmul(
                            sg[:], g_row[:, m * 128:(m + 1) * 128],
                            invsr[:], start=True, stop=True,
                            tile_position=(0, 0))
                        t1 = work.tile([128, S], f32, tag="scr2", name="t1")
                        nc.vector.tensor_tensor(
                            out=t1[:], in0=u_tiles[m][:], in1=sg[:],
                            op=ALU.mult)
                        ad = spp.tile([128, S], f32, tag="lnps", name="lnps")
                        nc.tensor.matmul(
                            ad[:], g_row[:, m * 128:(m + 1) * 128],
                            msn[:], start=True, stop=True,
                            tile_position=(0, 0))
                        nc.vector.scalar_tensor_tensor(
                            out=out_aps[m], in0=t1[:],
                            scalar=b_col[:, m:m + 1], in1=ad[:],
                            op0=ALU.add, op1=ALU.add)

                if stage <= 3:
                    break
                xT_new = [act.tile([128, N], f32r,
                                   tag=f"x{(l + 1) % 2}_{m}", name=f"x{(l + 1) % 2}_{m}")
                          for m in range(2)]
                for s in range(BL):
                    u_t = [work.tile([128, S], f32r, tag=f"u{m}", name=f"u{m}")
                           for m in range(2)]
                    for m in range(2):
                        po = pp.tile([128, S], f32, tag="mm", name="mm")
                        for k in range(2):
                            nc.tensor.matmul(
                                po[:], Wo[k][:, m * 128:(m + 1) * 128],
                                ctxT[s][k][:], start=(k == 0),
                                stop=(k == 1))
                        nc.vector.scalar_tensor_tensor(
                            out=u_t[m][:], in0=po[:], scalar=boe[m][:],
                            in1=xT[m][:, s * S:(s + 1) * S],
                            op0=ALU.add, op1=ALU.add)
                    y_t = [work.tile([128, S], f32r, tag=f"y{m}", name=f"y{m}")
                           for m in range(2)]
                    layer_norm(u_t, ln1g, ln1b,
                               [y_t[0][:], y_t[1][:]])
                    ff = []
                    for fc in range(8):
                        pf = pp.tile([128, S], f32, tag="mm", name="mm")
                        for k in range(2):
                            nc.tensor.matmul(
                                pf[:], W1[k][:, fc * 128:(fc + 1) * 128],
                                y_t[k][:], start=(k == 0), stop=(k == 1))
                        f_t = ffp.tile([128, S], f32r, tag="ff", name="ff")
                        nc.scalar.activation(
                            f_t[:], pf[:], AF.Relu,
                            bias=b1c[:, fc:fc + 1])
                        ff.append(f_t)
                    u2_t = [work.tile([128, S], f32r, tag=f"u{m}", name=f"u2{m}")
                            for m in range(2)]
                    for m in range(2):
                        p2 = spp.tile([128, S], f32, tag="ff2", name="ff2")
                        for fc in range(8):
                            nc.tensor.matmul(
                                p2[:], W2[fc][:, m * 128:(m + 1) * 128],
                                ff[fc][:], start=(fc == 0),
                                stop=(fc == 7))
                        nc.vector.scalar_tensor_tensor(
                            out=u2_t[m][:], in0=p2[:],
                            scalar=b2c[:, m:m + 1], in1=y_t[m][:],
                            op0=ALU.add, op1=ALU.add)
                    layer_norm(u2_t, ln2g, ln2b,
                               [xT_new[0][:, s * S:(s + 1) * S],
                                xT_new[1][:, s * S:(s + 1) * S]])
                xT = xT_new

            # ---------- classifier ----------
            for s in range(BL):
                pc = cpp.tile([26, S], f32, tag="cps", name="cps")
                for k in range(2):
                    nc.tensor.matmul(pc[:], clsW[k][:],
                                     xT[k][:, s * S:(s + 1) * S],
                                     start=(k == 0), stop=(k == 1))
                em = work.tile([NL, S], f32, tag="em", name="em")
                nc.scalar.activation(em[:], pc[0:NL, :], AF.Identity,
                                     bias=clsb[:, 0:1])
                nc.sync.dma_start(em_d[:, s * S:(s + 1) * S], em[:])

    nc.compile()
    return nc


def _pe_table():
    pos = np.arange(S, dtype=np.float32)[:, None]
    div = np.exp(np.arange(0, D, 2, dtype=np.float32)
                 * (-np.log(10000.0) / D))
    ang = pos * div
    pe = np.zeros((S, D), np.float32)
    pe[:, 0::2] = np.sin(ang)
    pe[:, 1::2] = np.cos(ang)
    return np.ascontiguousarray(pe.T)


def _prep_inputs(inputs, core, pe):
    f = lambda x: np.ascontiguousarray(np.asarray(x), dtype=np.float32)
    b0 = core * BL
    cids = np.asarray(inputs["char_ids"][b0:b0 + BL]).astype(
        np.int32).reshape(N)
    bids = np.asarray(inputs["bigram_ids"][b0:b0 + BL]).astype(
        np.int32).reshape(N)
    m = {
        "cids": np.ascontiguousarray(cids.reshape(NCH, 128).T),
        "bids": np.ascontiguousarray(bids.reshape(NCH, 128).T),
        "char_emb": f(inputs["char_emb"]),
        "bigram_emb": f(inputs["bigram_emb"]),
        "comb_W": f(inputs["comb_W"]), "comb_b": f(inputs["comb_b"]),
        "peT": pe,
        "rel_emb": f(inputs["rel_emb"]),
        "cls_W": f(inputs["cls_W"]), "cls_b": f(inputs["cls_b"]),
    }
    for k in ["Wq", "bq", "Wk", "bk", "Wv", "bv", "Wo", "bo", "ln1_g",
              "ln1_b", "W1", "b1", "W2", "b2", "ln2_g", "ln2_b"]:
        m[k] = f(inputs[k])
    return m


def kernel(**inputs):
    from concourse.bass_utils import run_bass_kernel_spmd

    if "nc" not in _cache:
        _cache["nc"] = _build()
    nc = _cache["nc"]
    pe = _pe_table()
    in_maps = [_prep_inputs(inputs, core, pe) for core in range(NCORES)]
    trace = bool(os.environ.get("KERNEL_TRACE"))
    res = run_bass_kernel_spmd(nc, in_maps,
                               core_ids=list(range(NCORES)),
                               trace=trace)
    _cache["last"] = res
    out = np.empty((B, S, NL), np.float32)
    for core in range(NCORES):
        em = res.results[core]["emT"]  # [NL, N]
        for s in range(BL):
            out[core * BL + s] = em[:, s * S:(s + 1) * S].T
    return out



# revision 11
# speedup vs baseline: 4.5872x; 4.5872x over previous
# Trainium2 Bass kernel for nn_FLAT_21242908246425 (2-layer relative-position
# transformer, B=16 S=512 D=256 H=8 DK=32 DFF=1024, V=8000 BV=50000 NL=25).
#
# Sharding: data-parallel over batch — 8 cores x 2 sequences each. No
# collectives. Each core runs the full network on its 2 sequences.
#
# Device layout: activations feature-major [D, N] in float32r (full-rate PE).
# Attention computes S^T = K Q^T per (seq, head) so the PV matmul needs no
# transpose of the softmax output. The relative-position term
# qrel[i, c(j-i)] is decomposed (after subtracting the softmax-invariant
# qrel[i,0]) into a 31-diagonal band + right-saturation. The band is
# realized with contiguous-run DMAs only: delta' is computed token-major
# (dpT = qt_chunk^T @ bd), its 31 band columns are written per token row
# into a persistent-zero DRAM row buffer (row i holds the band centered at
# col 143), read back as square/sliver tiles whose per-partition runs are
# contiguous, and transposed into the score PSUM by identity matmuls.
# Right-saturation is accumulated by constant-triangular x diag(d32)
# matmuls. Softmax Z comes free from a ones-column appended to V.
import os
import sys

import numpy as np

if "/opt/trn_rl_repo" not in sys.path:
    sys.path.insert(0, "/opt/trn_rl_repo")

B, S, D, H, DK, L, DFF, MAXREL, NL = 16, 512, 256, 8, 32, 2, 1024, 16, 25
V, BV = 8000, 50000
NCORES = 8
BL = B // NCORES          # sequences per core
N = BL * S                # local tokens (1024)
NCH = N // 128            # token chunks (8)
R = 2 * MAXREL + 1        # 33
SCALE = 1.0 / float(np.sqrt(DK))
NSCR = 4                  # rotating band row-buffer count
P2W = 288                 # per-head row width in the band row buffer
P2R = 4 * P2W             # full row width (4 heads)

_cache = {}


def _build():
    import concourse.bass as bass
    import concourse.mybir as mybir
    import concourse.tile as tile
    from concourse import bacc
    from concourse.masks import make_identity
    from concourse.tile_rust import add_dep_helper

    dt = mybir.dt
    ALU = mybir.AluOpType
    AF = mybir.ActivationFunctionType
    f32, f32r = dt.float32, dt.float32r

    nc = bacc.Bacc("TRN2", target_bir_lowering=False, debug=False,
                   num_devices=NCORES)

    def inp(name, shape, dtype=dt.float32):
        return nc.declare_dram_parameter(name, list(shape), dtype,
                                         isOutput=False)

    cids_d = inp("cids", [128, NCH], dt.int32)
    bids_d = inp("bids", [128, NCH], dt.int32)
    cemb_d = inp("char_emb", [V, D])
    bemb_d = inp("bigram_emb", [BV, D])
    combW_d = inp("comb_W", [2 * D, D])
    combb_d = inp("comb_b", [D])
    peT_d = inp("peT", [D, S])
    Wq_d = inp("Wq", [L, D, D]); bq_d = inp("bq", [L, D])
    Wk_d = inp("Wk", [L, D, D]); bk_d = inp("bk", [L, D])
    Wv_d = inp("Wv", [L, D, D]); bv_d = inp("bv", [L, D])
    Wo_d = inp("Wo", [L, D, D]); bo_d = inp("bo", [L, D])
    rel_d = inp("rel_emb", [L, R, DK])
    ln1g_d = inp("ln1_g", [L, D]); ln1b_d = inp("ln1_b", [L, D])
    ln2g_d = inp("ln2_g", [L, D]); ln2b_d = inp("ln2_b", [L, D])
    W1_d = inp("W1", [L, D, DFF]); b1_d = inp("b1", [L, DFF])
    W2_d = inp("W2", [L, DFF, D]); b2_d = inp("b2", [L, D])
    clsW_d = inp("cls_W", [D, NL]); clsb_d = inp("cls_b", [NL])

    em_d = nc.declare_dram_parameter("emT", [NL, N], dt.float32,
                                     isOutput=True)
    p2b = [nc.dram_tensor(f"p2_{i}", [S, P2R], dt.float32)
           for i in range(NSCR)]

    with tile.TileContext(nc) as tc:
        import contextlib
        ctx = contextlib.ExitStack()
        with ctx:
            consts = ctx.enter_context(tc.tile_pool(name="consts", bufs=1))
            wpool = ctx.enter_context(tc.tile_pool(name="weights", bufs=1))
            act = ctx.enter_context(tc.tile_pool(name="acts", bufs=1))
            work = ctx.enter_context(tc.tile_pool(name="work", bufs=2))
            ffp = ctx.enter_context(tc.tile_pool(name="ffp", bufs=8))
            ptp = ctx.enter_context(tc.tile_pool(name="ptp", bufs=3))
            wtp = ctx.enter_context(tc.tile_pool(name="wtp", bufs=2))
            wtp2 = ctx.enter_context(tc.tile_pool(name="wtp2", bufs=3))
            rowp = ctx.enter_context(tc.tile_pool(name="rowp", bufs=6))
            rowp2 = ctx.enter_context(tc.tile_pool(name="rowp2", bufs=2))
            # PSUM: 8 banks total: mm(2) + stps(2) + cps(2) + lnps(1) + ff2(1)
            pp = ctx.enter_context(
                tc.tile_pool(name="pp", bufs=2, space="PSUM"))
            cpp = ctx.enter_context(
                tc.tile_pool(name="cpp", bufs=2, space="PSUM"))
            spp = ctx.enter_context(
                tc.tile_pool(name="spp", bufs=1, space="PSUM"))

            # ---------- constants ----------
            ident = consts.tile([128, 128], f32, tag="ident", name="ident")
            make_identity(nc, ident[:])

            ones_row_f = consts.tile([1, 128], f32, tag="ones_row_f", name="ones_row_f")
            nc.gpsimd.memset(ones_row_f[:], 1.0)
            ones_row = consts.tile([1, 128], f32r, tag="ones_row", name="ones_row")
            nc.vector.tensor_copy(ones_row[:], ones_row_f[:])
            ones_col_f = consts.tile([128, 1], f32, tag="ones_col_f", name="ones_col_f")
            nc.gpsimd.memset(ones_col_f[:], 1.0)
            ones_col = consts.tile([128, 1], f32r, tag="ones_col", name="ones_col")
            nc.vector.tensor_copy(ones_col[:], ones_col_f[:])

            onz_f = consts.tile([128, 2], f32, tag="onz_f", name="onz_f")
            nc.gpsimd.memset(onz_f[:, 0:1], 1.0)
            nc.gpsimd.memset(onz_f[:, 1:2], 0.0)
            onz_row = consts.tile([128, 2], f32r, tag="onz", name="onz")
            nc.vector.tensor_copy(onz_row[:], onz_f[:])

            zo_f = consts.tile([2, 128], f32, tag="zo_f", name="zo_f")
            nc.gpsimd.memset(zo_f[:], 0.0)
            nc.gpsimd.affine_select(
                out=zo_f[:], in_=zo_f[:], compare_op=ALU.not_equal,
                fill=1.0, base=0, pattern=[[0, 128]], channel_multiplier=1)
            zo_row = consts.tile([2, 128], f32r, tag="zo", name="zo")
            nc.vector.tensor_copy(zo_row[:], zo_f[:])

            eps_t = consts.tile([1, 1], f32, tag="eps", name="eps")
            nc.gpsimd.memset(eps_t[:], 1e-5)

            # M^T [33, 32]: row k, col r': 1 if k==r'+1, -1 if k==0
            mt_f = consts.tile([R, DK], f32, tag="mt_f", name="mt_f")
            nc.gpsimd.memset(mt_f[:], 0.0)
            nc.gpsimd.affine_select(
                out=mt_f[:], in_=mt_f[:], compare_op=ALU.not_equal,
                fill=1.0, base=-1, pattern=[[-1, DK]], channel_multiplier=1)
            nc.gpsimd.affine_select(
                out=mt_f[:], in_=mt_f[:], compare_op=ALU.not_equal,
                fill=-1.0, base=0, pattern=[[0, DK]], channel_multiplier=1)
            mt = consts.tile([R, DK], f32r, tag="mt", name="mt")
            nc.vector.tensor_copy(mt[:], mt_f[:])

            # identity in f32r for transpose-via-matmul rhs
            identr = consts.tile([128, 128], f32r, tag="identr",
                                 name="identr")
            nc.vector.tensor_copy(identr[:], ident[:])

            # triangular saturation masks (lhsT layout [p=i_local, m=jj]):
            # triA[p, m] = 1 if p <= m - 16 ; triB[p, m] = 1 if p <= m + 112
            triA_f = consts.tile([128, 128], f32, tag="triA_f", name="triA_f")
            nc.gpsimd.memset(triA_f[:], 0.0)
            nc.gpsimd.affine_select(
                out=triA_f[:], in_=triA_f[:], compare_op=ALU.is_gt,
                fill=1.0, base=16, pattern=[[-1, 128]], channel_multiplier=1)
            triA = consts.tile([128, 128], f32r, tag="triA", name="triA")
            nc.vector.tensor_copy(triA[:], triA_f[:])
            triB_f = consts.tile([128, 128], f32, tag="triB_f", name="triB_f")
            nc.gpsimd.memset(triB_f[:], 0.0)
            nc.gpsimd.affine_select(
                out=triB_f[:], in_=triB_f[:], compare_op=ALU.is_gt,
                fill=1.0, base=-112, pattern=[[-1, 128]],
                channel_multiplier=1)
            triB = consts.tile([128, 128], f32r, tag="triB", name="triB")
            nc.vector.tensor_copy(triB[:], triB_f[:])
            ones128_f = consts.tile([128, 128], f32, tag="o128f",
                                    name="o128f")
            nc.gpsimd.memset(ones128_f[:], 1.0)
            ones128 = consts.tile([128, 128], f32r, tag="o128", name="o128")
            nc.vector.tensor_copy(ones128[:], ones128_f[:])

            # persistent-zero band row buffers: only cols [128,159) of each
            # 288-wide head section are ever rewritten; the rest stays 0.
            ztw = consts.tile([128, P2R], f32, tag="ztw", name="ztw")
            nc.gpsimd.memset(ztw[:], 0.0)
            for i in range(NSCR):
                for k in range(S // 128):
                    nc.sync.dma_start(p2b[i][k * 128:(k + 1) * 128, :],
                                      ztw[:])

            peT = []
            for m in range(2):
                t = consts.tile([128, S], f32, tag=f"peT{m}", name=f"peT{m}")
                nc.sync.dma_start(t[:], peT_d[m * 128:(m + 1) * 128, :])
                peT.append(t)

            def load_round(dram_ap, shape, tag, scale=1.0):
                raw = work.tile(list(shape), f32, tag="ldraw")
                nc.sync.dma_start(raw[:], dram_ap)
                t = wpool.tile(list(shape), f32r, tag=tag)
                if scale == 1.0:
                    nc.vector.tensor_copy(t[:], raw[:])
                else:
                    nc.scalar.mul(t[:], raw[:], scale)
                return t

            def load_col(dram_1d, n, tag, scale=1.0):
                cols = (n + 127) // 128
                p = min(n, 128)
                t = consts.tile([p, cols], f32, tag=tag)
                if n >= 128:
                    nc.sync.dma_start(
                        t[:], dram_1d.rearrange("(c p) -> p c", p=128))
                else:
                    nc.sync.dma_start(t[:], dram_1d.rearrange("(p o) -> p o", o=1))
                if scale != 1.0:
                    nc.scalar.mul(t[:], t[:], scale)
                return t

            def load_row_r(dram_1d, n, tag):
                raw = work.tile([1, n], f32, tag="ldrow", name="ldrow")
                nc.sync.dma_start(raw[:], dram_1d.rearrange("(o n) -> o n", o=1))
                t = consts.tile([1, n], f32r, tag=tag)
                nc.vector.tensor_copy(t[:], raw[:])
                return t

            combW = [load_round(combW_d[k * 128:(k + 1) * 128, :],
                                [128, D], f"combW{k}") for k in range(4)]
            combb = load_col(combb_d[:], D, "combb")
            clsW = []
            for k in range(2):
                craw = work.tile([128, 26], f32, tag="ldraw",
                                 name=f"clsraw{k}")
                nc.gpsimd.memset(craw[:], 0.0)
                nc.sync.dma_start(craw[:, 0:NL],
                                  clsW_d[k * 128:(k + 1) * 128, :])
                t = wpool.tile([128, 26], f32r, tag=f"clsW{k}",
                               name=f"clsW{k}")
                nc.vector.tensor_copy(t[:], craw[:])
                clsW.append(t)
            clsb = load_col(clsb_d[:], NL, "clsb")

            # ---------- embeddings -> xcatT [4][128, N] f32r ----------
            _xct = ["q0", "q1", "k0", "k1"]
            xc = [act.tile([128, N], f32r, tag=_xct[k], name=f"xc{k}")
                  for k in range(4)]
            cids = consts.tile([128, NCH], dt.int32, tag="cids", name="cids")
            bids = consts.tile([128, NCH], dt.int32, tag="bids", name="bids")
            nc.sync.dma_start(cids[:], cids_d[:])
            nc.sync.dma_start(bids[:], bids_d[:])
            for c in range(NCH):
                for tbl, (ids, emb) in enumerate(
                        [(cids, cemb_d), (bids, bemb_d)]):
                    g = work.tile([128, D], f32, tag="gath", name="gath")
                    nc.gpsimd.indirect_dma_start(
                        out=g[:], out_offset=None, in_=emb[:],
                        in_offset=bass.IndirectOffsetOnAxis(
                            ap=ids[:, c:c + 1], axis=0))
                    for dc in range(2):
                        ptt = pp.tile([128, 128], f32, tag="mm", name="mm")
                        nc.tensor.transpose(
                            ptt[:], g[:, dc * 128:(dc + 1) * 128], ident[:])
                        nc.scalar.copy(
                            xc[tbl * 2 + dc][:, c * 128:(c + 1) * 128],
                            ptt[:])

            # ---------- comb + pe -> xT [2][128, N] f32r ----------
            xT = [act.tile([128, N], f32r, tag=f"x0_{m}", name=f"x0_{m}") for m in range(2)]
            for m in range(2):
                for s in range(BL):
                    ps = pp.tile([128, S], f32, tag="mm", name="mm")
                    for k in range(4):
                        nc.tensor.matmul(
                            ps[:], combW[k][:, m * 128:(m + 1) * 128],
                            xc[k][:, s * S:(s + 1) * S],
                            start=(k == 0), stop=(k == 3))
                    nc.vector.scalar_tensor_tensor(
                        out=xT[m][:, s * S:(s + 1) * S], in0=ps[:],
                        scalar=combb[:, m:m + 1], in1=peT[m][:],
                        op0=ALU.add, op1=ALU.add)

            # ---------- layers ----------
            bh_idx = 0
            for l in range(L):
                Wq = [load_round(Wq_d[l, k * 128:(k + 1) * 128, :],
                                 [128, D], "Wq%d" % k, scale=SCALE)
                      for k in range(2)]
                Wk_ = [load_round(Wk_d[l, k * 128:(k + 1) * 128, :],
                                  [128, D], "Wkk%d" % k) for k in range(2)]
                Wv = [load_round(Wv_d[l, k * 128:(k + 1) * 128, :],
                                 [128, D], "Wv%d" % k) for k in range(2)]
                Wo = [load_round(Wo_d[l, k * 128:(k + 1) * 128, :],
                                 [128, D], "Wo%d" % k) for k in range(2)]
                W1 = [load_round(W1_d[l, k * 128:(k + 1) * 128, :],
                                 [128, DFF], "W1_%d" % k) for k in range(2)]
                W2 = [load_round(W2_d[l, k * 128:(k + 1) * 128, :],
                                 [128, D], "W2_%d" % k) for k in range(8)]
                bq = load_col(bq_d[l, :], D, "bqc", scale=SCALE)
                bk = load_col(bk_d[l, :], D, "bkc")
                bvc = load_round(bv_d[l, :].rearrange("(c p) -> p c", p=128),
                                 [128, 2], "bvc")
                boc = load_col(bo_d[l, :], D, "boc")
                b1c = load_col(b1_d[l, :], DFF, "b1c")
                b2c = load_col(b2_d[l, :], D, "b2c")
                ln1g = load_row_r(ln1g_d[l, :], D, "ln1g")
                ln1b = load_col(ln1b_d[l, :], D, "ln1b")
                ln2g = load_row_r(ln2g_d[l, :], D, "ln2g")
                ln2b = load_col(ln2b_d[l, :], D, "ln2b")

                # relM = rel_emb_eff^T @ M^T  [DK, 32]; r' col = delta'_{r'+1}
                rel_sb = load_round(rel_d[l, :, :], [R, DK], "rel",
                                    scale=float(np.sqrt(DK)))
                relM_ps = spp.tile([DK, DK], f32, tag="lnps", name="lnps")
                nc.tensor.matmul(relM_ps[:], rel_sb[:], mt[:],
                                 start=True, stop=True)
                relM = wpool.tile([DK, DK], f32r, tag="relM", name="relM")
                nc.scalar.copy(relM[:], relM_ps[:])
                bd_f = wpool.tile([128, 128], f32, tag="bdf", name="bdf")
                nc.gpsimd.memset(bd_f[:], 0.0)
                bd = wpool.tile([128, 128], f32r, tag="bd", name="bd")
                nc.vector.tensor_copy(bd[:], bd_f[:])
                for hq in range(4):
                    nc.vector.tensor_copy(
                        bd[hq * 32:(hq + 1) * 32,
                           hq * 32:(hq + 1) * 32], relM[:])

                # bo_eff = bo + Wo^T bv
                bv2 = []
                for k in range(2):
                    t = wpool.tile([128, 2], f32r, tag=f"bv2_{k}",
                                   name=f"bv2_{k}")
                    nc.vector.tensor_copy(t[:, 0:1], bvc[:, k:k + 1])
                    nc.vector.tensor_copy(t[:, 1:2], onz_row[:, 1:2])
                    bv2.append(t)
                boe = []
                for m in range(2):
                    pb = spp.tile([128, 2], f32, tag="lnps", name="lnps")
                    for k in range(2):
                        nc.tensor.matmul(
                            pb[:], Wo[k][:, m * 128:(m + 1) * 128],
                            bv2[k][:], start=(k == 0), stop=(k == 1))
                    t = wpool.tile([128, 1], f32, tag="boe%d" % m, name="boe%d" % m)
                    nc.vector.tensor_add(t[:], pb[:, 0:1], boc[:, m:m + 1])
                    boe.append(t)

                # QKV projections
                qt = [act.tile([128, N], f32r, tag=f"q{m}", name=f"q{m}")
                      for m in range(2)]
                kt = [act.tile([128, N], f32r, tag=f"k{m}", name=f"k{m}")
                      for m in range(2)]
                for m in range(2):
                    for s in range(BL):
                        psq = pp.tile([128, S], f32, tag="mm", name="mm")
                        for k in range(2):
                            nc.tensor.matmul(
                                psq[:], Wq[k][:, m * 128:(m + 1) * 128],
                                xT[k][:, s * S:(s + 1) * S],
                                start=(k == 0), stop=(k == 1))
                        nc.scalar.activation(
                            qt[m][:, s * S:(s + 1) * S], psq[:],
                            AF.Identity, bias=bq[:, m:m + 1])
                        psk = pp.tile([128, S], f32, tag="mm", name="mm")
                        for k in range(2):
                            nc.tensor.matmul(
                                psk[:], Wk_[k][:, m * 128:(m + 1) * 128],
                                xT[k][:, s * S:(s + 1) * S],
                                start=(k == 0), stop=(k == 1))
                        nc.scalar.activation(
                            kt[m][:, s * S:(s + 1) * S], psk[:],
                            AF.Identity, bias=bk[:, m:m + 1])

                # V token-major with ones column: vaug[c] [128, 8*33]
                vaug = []
                for c in range(NCH):
                    pv = pp.tile([128, D], f32, tag="mm", name="mm")
                    for k in range(2):
                        nc.tensor.matmul(
                            pv[:],
                            xT[k][:, c * 128:(c + 1) * 128], Wv[k][:],
                            start=(k == 0), stop=(k == 1))
                    va = act.tile([128, 272], f32r, tag=f"va{c}", name=f"va{c}")
                    nc.vector.tensor_copy(
                        va[:].rearrange("p (h c) -> p h c", c=34)[:, :, 0:32],
                        pv[:].rearrange("p (h c) -> p h c", c=32))
                    tail_view = va[:].rearrange(
                        "p (h c) -> p h c", c=34)[:, :, 32:34]
                    nc.vector.tensor_copy(
                        tail_view,
                        onz_row[:].unsqueeze(1).to_broadcast([128, 8, 2]))
                    vaug.append(va)

                # attention
                ctxT = [[act.tile([128, S], f32r, tag=f"ctx{s}{m}", name=f"ctx{s}{m}")
                         for m in range(2)] for s in range(BL)]
                for s in range(BL):
                    for hc in range(2):
                        # dpT token-major: dpt[ii, c*128 + hl*32 + t]
                        #   = delta'[t+1, 128c+ii] (t=31 -> d32)
                        dpt_ps = pp.tile([128, S], f32, tag="mm", name="mm")
                        for c in range(4):
                            nc.tensor.matmul(
                                dpt_ps[:, c * 128:(c + 1) * 128],
                                qt[hc][:, s * S + c * 128:
                                       s * S + (c + 1) * 128],
                                bd[:], start=True, stop=True,
                                skip_group_check=True)
                        dpt = act.tile([128, S], f32r, tag=f"dpt{hc}",
                                       name=f"dpt{hc}")
                        nc.scalar.copy(dpt[:], dpt_ps[:])
                        # band write: row i gets its 31 band values at
                        # cols [128,159) of head section hl
                        buf = p2b[bh_idx % NSCR]
                        bh_idx += 1
                        p2flat = buf[:, :].rearrange("a b -> (a b)")
                        for c in range(4):
                            bsrc = dpt[:, c * 128:(c + 1) * 128].rearrange(
                                "p (h t) -> p h t", h=4)[:, :, 0:31]
                            bdst = bass.AP(
                                p2flat.tensor, c * 128 * P2R + 128,
                                [[P2R, 128], [P2W, 4], [1, 31]])
                            nc.sync.dma_start(bdst, bsrc.bitcast(f32))
                        for hl in range(4):
                            h = hc * 4 + hl
                            # diag(d32 chunk) for saturation matmuls
                            diagt = []
                            for c in range(4):
                                dg = wtp.tile([128, 128], f32r,
                                              tag=f"dg{c}", name=f"dg{c}")
                                nc.vector.tensor_scalar(
                                    out=dg[:], in0=identr[:],
                                    scalar1=dpt[:, c * 128 + hl * 32 + 31:
                                                c * 128 + hl * 32 + 32]
                                    .bitcast(f32),
                                    scalar2=None, op0=ALU.mult)
                                diagt.append(dg)
                            # band tiles: per-partition-contiguous reads
                            bsq, bslL, bslR = [], [None] * 4, [None] * 4
                            for u in range(4):
                                tq = wtp2.tile([128, 128], f32r,
                                               tag=f"bsq{u}", name=f"bsq{u}")
                                ap = bass.AP(
                                    p2flat.tensor,
                                    128 * u * P2R + hl * P2W + 143,
                                    [[P2R - 1, 128], [1, 128]])
                                nc.sync.dma_start(tq[:].bitcast(f32), ap)
                                bsq.append(tq)
                                if u >= 1:
                                    tL = wtp2.tile([16, 128], f32r,
                                                   tag=f"bsL{u}",
                                                   name=f"bsL{u}")
                                    ap = bass.AP(
                                        p2flat.tensor,
                                        (128 * u - 16) * P2R
                                        + hl * P2W + 159,
                                        [[P2R - 1, 16], [1, 128]])
                                    nc.sync.dma_start(
                                        tL[:].bitcast(f32), ap)
                                    bslL[u] = tL
                                if u <= 2:
                                    tR = wtp2.tile([16, 128], f32r,
                                                   tag=f"bsR{u}",
                                                   name=f"bsR{u}")
                                    ap = bass.AP(
                                        p2flat.tensor,
                                        (128 * u + 128) * P2R
                                        + hl * P2W + 15,
                                        [[P2R - 1, 16], [1, 128]])
                                    nc.sync.dma_start(
                                        tR[:].bitcast(f32), ap)
                                    bslR[u] = tR
                            cps = cpp.tile([34, S], f32, tag="cps",
                                           name="cps")
                            for u in range(4):
                                st = pp.tile([128, S], f32, tag="stps",
                                             name="stps")
                                nc.tensor.matmul(
                                    st[:],
                                    kt[hc][hl * 32:(hl + 1) * 32,
                                           s * S + u * 128:
                                           s * S + u * 128 + 128],
                                    qt[hc][hl * 32:(hl + 1) * 32,
                                           s * S:(s + 1) * S],
                                    start=True, stop=False,
                                    tile_position=(hl * 32, 0))
                                # band: transpose-accumulate via identity
                                mms = [(bsq[u], identr[:],
                                        (u * 128, u * 128 + 128))]
                                if bslL[u] is not None:
                                    mms.append((bslL[u], identr[0:16, 0:16],
                                                (u * 128 - 16, u * 128)))
                                if bslR[u] is not None:
                                    mms.append((bslR[u], identr[0:16, 0:16],
                                                (u * 128 + 128,
                                                 u * 128 + 144)))
                                # right-saturation: tri x diag(d32)
                                mms.append((triA, diagt[u][:],
                                            (u * 128, u * 128 + 128)))
                                if u >= 1:
                                    mms.append((triB, diagt[u - 1][:],
                                                ((u - 1) * 128, u * 128)))
                                for c in range(u - 1):
                                    mms.append((ones128, diagt[c][:],
                                                (c * 128, c * 128 + 128)))
                                for q, (lh, rh, (c0, c1)) in enumerate(mms):
                                    nc.tensor.matmul(
                                        st[:, c0:c1], lh[:], rh,
                                        start=False,
                                        stop=(q == len(mms) - 1),
                                        skip_group_check=True)
                                pt = ptp.tile([128, S], f32r, tag="pt",
                                              name="pt")
                                nc.scalar.activation(pt[:], st[:], AF.Exp)
                                nc.tensor.matmul(
                                    cps[:],
                                    vaug[s * 4 + u][:, h * 34:(h + 1) * 34],
                                    pt[:], start=(u == 0), stop=(u == 3),
                                    tile_position=(0, 0))
                            rz = rowp2.tile([1, S], f32, tag="rz", name="rz")
                            nc.vector.reciprocal(rz[:], cps[32:33, :])
                            rzb = wtp.tile([32, S], f32, tag="rzb",
                                           name="rzb")
                            nc.gpsimd.partition_broadcast(
                                rzb[:], rz[:], channels=32)
                            nc.vector.tensor_tensor(
                                out=ctxT[s][hc][hl * 32:(hl + 1) * 32, :],
                                in0=cps[0:32, :], in1=rzb[:], op=ALU.mult)

                def layer_norm(u_tiles, g_row, b_col, out_aps):
                    psu = spp.tile([1, S], f32, tag="lnps", name="lnps")
                    for m in range(2):
                        nc.tensor.matmul(psu[:], ones_col[:],
                                         u_tiles[m][:],
                                         start=(m == 0), stop=(m == 1),
                                         tile_position=(0, 0))
                    mm = rowp.tile([1, S], f32, tag="lnrow", name="mmu")
                    nc.vector.tensor_scalar(
                        out=mm[:], in0=psu[:], scalar1=1.0 / D,
                        scalar2=None, op0=ALU.mult)
                    usq = [work.tile([128, S], f32r, tag="scr2", name="usq")
                           for _ in range(2)]
                    for m in range(2):
                        nc.scalar.square(usq[m][:], u_tiles[m][:])
                    psq2 = spp.tile([1, S], f32, tag="lnps", name="lnps")
                    for m in range(2):
                        nc.tensor.matmul(psq2[:], ones_col[:], usq[m][:],
                                         start=(m == 0), stop=(m == 1),
                                         tile_position=(0, 0))
                    mm2 = rowp.tile([1, S], f32, tag="lnrow", name="mm2")
                    nc.vector.tensor_tensor(out=mm2[:], in0=mm[:],
                                            in1=mm[:], op=ALU.mult)
                    var = rowp.tile([1, S], f32, tag="lnrow", name="var")
                    nc.vector.scalar_tensor_tensor(
                        out=var[:], in0=psq2[:], scalar=1.0 / D,
                        in1=mm2[:], op0=ALU.mult, op1=ALU.subtract)
                    sd = rowp.tile([1, S], f32, tag="lnrow", name="sd")
                    nc.scalar.activation(sd[:], var[:], AF.Sqrt,
                                         bias=eps_t[:])
                    invs = rowp.tile([1, S], f32, tag="lnrow", name="invs")
                    nc.vector.reciprocal(invs[:], sd[:])
                    invsr = rowp.tile([1, S], f32r, tag="lnrow", name="invsr")
                    nc.vector.tensor_copy(invsr[:], invs[:])
                    msn = rowp.tile([1, S], f32r, tag="lnrow", name="msn")
                    nc.vector.scalar_tensor_tensor(
                        out=msn[:], in0=mm[:], scalar=-1.0, in1=invs[:],
                        op0=ALU.mult, op1=ALU.mult)
                    for m in range(2):
                        sg = spp.tile([128, S], f32, tag="lnps", name="lnps")
                        nc.tensor.matmul(
                            sg[:], g_row[:, m * 128:(m + 1) * 128],
                            invsr[:], start=True, stop=True,
                            tile_position=(0, 0))
                        t1 = work.tile([128, S], f32, tag="scr2", name="t1")
                        nc.vector.tensor_tensor(
                            out=t1[:], in0=u_tiles[m][:], in1=sg[:],
                            op=ALU.mult)
                        ad = spp.tile([128, S], f32, tag="lnps", name="lnps")
                        nc.tensor.matmul(
                            ad[:], g_row[:, m * 128:(m + 1) * 128],
                            msn[:], start=True, stop=True,
                            tile_position=(0, 0))
                        nc.vector.scalar_tensor_tensor(
                            out=out_aps[m], in0=t1[:],
                            scalar=b_col[:, m:m + 1], in1=ad[:],
                            op0=ALU.add, op1=ALU.add)

                xT_new = [act.tile([128, N], f32r,
                                   tag=f"x{(l + 1) % 2}_{m}", name=f"x{(l + 1) % 2}_{m}")
                          for m in range(2)]
                for s in range(BL):
                    u_t = [work.tile([128, S], f32r, tag=f"u{m}", name=f"u{m}")
                           for m in range(2)]
                    for m in range(2):
                        po = pp.tile([128, S], f32, tag="mm", name="mm")
                        for k in range(2):
                            nc.tensor.matmul(
                                po[:], Wo[k][:, m * 128:(m + 1) * 128],
                                ctxT[s][k][:], start=(k == 0),
                                stop=(k == 1))
                        nc.vector.scalar_tensor_tensor(
                            out=u_t[m][:], in0=po[:], scalar=boe[m][:],
                            in1=xT[m][:, s * S:(s + 1) * S],
                            op0=ALU.add, op1=ALU.add)
                    y_t = [work.tile([128, S], f32r, tag=f"y{m}", name=f"y{m}")
                           for m in range(2)]
                    layer_norm(u_t, ln1g, ln1b,
                               [y_t[0][:], y_t[1][:]])
                    ff = []
                    for fc in range(8):
                        pf = pp.tile([128, S], f32, tag="mm", name="mm")
                        for k in range(2):
                            nc.tensor.matmul(
                                pf[:], W1[k][:, fc * 128:(fc + 1) * 128],
                                y_t[k][:], start=(k == 0), stop=(k == 1))
                        f_t = ffp.tile([128, S], f32r, tag="ff", name="ff")
                        nc.scalar.activation(
                            f_t[:], pf[:], AF.Relu,
                            bias=b1c[:, fc:fc + 1])
                        ff.append(f_t)
                    u2_t = [work.tile([128, S], f32r, tag=f"u{m}", name=f"u2{m}")
                            for m in range(2)]
                    for m in range(2):
                        p2 = spp.tile([128, S], f32, tag="ff2", name="ff2")
                        for fc in range(8):
                            nc.tensor.matmul(
                                p2[:], W2[fc][:, m * 128:(m + 1) * 128],
                                ff[fc][:], start=(fc == 0),
                                stop=(fc == 7))
                        nc.vector.scalar_tensor_tensor(
                            out=u2_t[m][:], in0=p2[:],
                            scalar=b2c[:, m:m + 1], in1=y_t[m][:],
                            op0=ALU.add, op1=ALU.add)
                    layer_norm(u2_t, ln2g, ln2b,
                               [xT_new[0][:, s * S:(s + 1) * S],
                                xT_new[1][:, s * S:(s + 1) * S]])
                xT = xT_new

            # ---------- classifier ----------
            for s in range(BL):
                pc = cpp.tile([26, S], f32, tag="cps", name="cps")
                for k in range(2):
                    nc.tensor.matmul(pc[:], clsW[k][:],
                                     xT[k][:, s * S:(s + 1) * S],
                                     start=(k == 0), stop=(k == 1))
                em = work.tile([NL, S], f32, tag="em", name="em")
                nc.scalar.activation(em[:], pc[0:NL, :], AF.Identity,
                                     bias=clsb[:, 0:1])
                nc.sync.dma_start(em_d[:, s * S:(s + 1) * S], em[:])

    nc.compile()
    return nc


def _pe_table():
    pos = np.arange(S, dtype=np.float32)[:, None]
    div = np.exp(np.arange(0, D, 2, dtype=np.float32)
                 * (-np.log(10000.0) / D))
    ang = pos * div
    pe = np.zeros((S, D), np.float32)
    pe[:, 0::2] = np.sin(ang)
    pe[:, 1::2] = np.cos(ang)
    return np.ascontiguousarray(pe.T)


def _prep_inputs(inputs, core, pe):
    f = lambda x: np.ascontiguousarray(np.asarray(x), dtype=np.float32)
    b0 = core * BL
    cids = np.asarray(inputs["char_ids"][b0:b0 + BL]).astype(
        np.int32).reshape(N)
    bids = np.asarray(inputs["bigram_ids"][b0:b0 + BL]).astype(
        np.int32).reshape(N)
    m = {
        "cids": np.ascontiguousarray(cids.reshape(NCH, 128).T),
        "bids": np.ascontiguousarray(bids.reshape(NCH, 128).T),
        "char_emb": f(inputs["char_emb"]),
        "bigram_emb": f(inputs["bigram_emb"]),
        "comb_W": f(inputs["comb_W"]), "comb_b": f(inputs["comb_b"]),
        "peT": pe,
        "rel_emb": f(inputs["rel_emb"]),
        "cls_W": f(inputs["cls_W"]), "cls_b": f(inputs["cls_b"]),
    }
    for k in ["Wq", "bq", "Wk", "bk", "Wv", "bv", "Wo", "bo", "ln1_g",
              "ln1_b", "W1", "b1", "W2", "b2", "ln2_g", "ln2_b"]:
        m[k] = f(inputs[k])
    return m


def kernel(**inputs):
    from concourse.bass_utils import run_bass_kernel_spmd

    if "nc" not in _cache:
        _cache["nc"] = _build()
    nc = _cache["nc"]
    pe = _pe_table()
    in_maps = [_prep_inputs(inputs, core, pe) for core in range(NCORES)]
    trace = bool(os.environ.get("KERNEL_TRACE"))
    res = run_bass_kernel_spmd(nc, in_maps,
                               core_ids=list(range(NCORES)),
                               trace=trace)
    _cache["last"] = res
    out = np.empty((B, S, NL), np.float32)
    for core in range(NCORES):
        em = res.results[core]["emT"]  # [NL, N]
        for s in range(BL):
            out[core * BL + s] = em[:, s * S:(s + 1) * S].T
    return out

